# revision 21
# baseline (speedup 1.0000x reference)
"""Trainium2 Bass kernel: EnhancedSpikingNeuron (LIF, soft reset) forward.

Reference semantics (per element chain (b, d), sequential over t):
    mem = beta * mem + (x[b, t, d] + homeo_i)
    s   = (mem - 1.0 > 0) ? 1.0 : 0.0
    mem = mem - s
Output = spikes [B, T, D] float32.

Implementation notes
--------------------
TIME-sharded across the 8 cores (v1 was batch-sharded at ~502us): core c
owns output steps [256c, 256c+256) and recomputes a W=128-step warm-up
from zero state. beta=0.9 contracts state, so the warm-up resynchronizes
the membrane; measured rel err ~8e-3 vs the bit-exact reference (gate is
2e-2). Core 0's warm-up input is zero-padded (zero input holds zero
state, so its output is exact). Sequential chain hops drop 2048 -> 384,
and each hop carries ALL B*D = 16384 chains ([128 part, 128 free] per
step), amortizing per-instruction overhead.

Layout: partition p = b*8 + (d>>7), free j = d&127, time-major per core
(host pre/post-transposes to/from this "pmaj" layout), so every DMA is a
contiguous run at line rate.

The chain runs on DVE with a hand-built 2-timestep fused custom op
(LIF_STEP2_ANT): a 3-uOp FSM alternating per element — uOp A consumes
(u_t, x1) and computes v = u_{t+1} in ALU stages 0-3, bypassing v
through stages 4-7; uOp B consumes x2 and computes u_{t+2} in stages
4-7, reading v via same-stage CURR_ALU_OUT feedback (the scan
mechanism). Both membrane values stream out through one [P, FD, 2]
strided AP into the U tile, so rounding is op-for-op identical to the
reference (measured bit-exact). Per-instruction cost = (2*sub + 58
SBUF-access cycles) at 0.96 GHz; interleave=2 splits FD into two
64-wide sub-chains so the RAW dependency latency of one sub-chain hides
under the other's exec. DVE span/rep = 384 * 193.7ns ~ 74.4us — the
per-element floor for an fp32 2-source DVE op (1 elem/cycle).

Warm-up x for steps [0,64) ships as fp16 (boundary-state perturbation
~4e-7, negligible vs the 2e-6 zero-init residual): halves the first two
block loads so the DMA ring (~6.3us per fp32 block vs ~6.2us DVE per
block) gains margin and the head shrinks.

Spike extraction: ONE ACT pass per K-block — activation Sign with
scale=-1, bias=1 gives sign(1-u) in {-1,0,+1} (exact in fp8e4); the
host maps (val < 0) -> spike. (ACT has no is_gt; the old 2-pass
sign+relu lagged DVE per-block and serialized a ~20us tail. Pool-engine
is_gt measured ~16ns/elem on HW — firmware path, unusable.) Loads ride
the SP HWDGE ring, stores the ACT ring.

reps>1 wraps the body in a For_i hardware loop; each For_i iteration
ends in an all-engine barrier + semaphore reset, which serializes
head/chain/tail. unroll=U emits U rep-bodies per iteration so reps
pipeline through the tile-pool semaphores (cross-rep overlap of loads,
extraction, stores with the next rep's chain). Measured (two-large-R
slope, same session): unroll 1/2/4/8 -> 104/98/77-81/73 us per rep;
DVE-span model floor 74.4us. interleave=1 (192 dep-distance-1 ops)
measured ~140us/rep — same-engine RAW hops cost ~300-400ns exposed on
HW, so keep dep distance 2. SP-ring loads measured at ~458 GB/s
(25.2MB/rep = 55us, hidden under DVE); splitting loads across
SP+gpsimd(SWDGE) rings measured SLOWER — keep all loads on SP HWDGE.
Pool-engine compute (tensor_scalar) measured ~16ns/elem — never use.
"""

import functools
from contextlib import ExitStack

import numpy as np

import concourse.bass as bass
import concourse.bacc as bacc
import concourse.mybir as mybir
import concourse.tile as tile
from concourse.bass_utils import run_bass_kernel_spmd


# --------------------------------------------------------------------------
# Custom DVE ops
# --------------------------------------------------------------------------

def _register_lif_op():
    """Register the fused 1-step LIF custom DVE op (idempotent).

    One 4-stage DVE instruction per timestep:
        u' = (u - (u > 1.0)) * beta + x'
    Each stage rounds fp32, reproducing the reference's op-for-op
    rounding exactly ((u - 1 > 0) <=> (u > 1) in fp32 near 1.0).
    """
    from concourse import dve_ops
    from concourse.dve_spec import Spec, Src0, Src1, C0, C1

    for op in dve_ops.OPS:
        if op.name == "LIF_STEP_ANT":
            return op

    def _ref(in0, in1, s0, s1, imm2):
        s = (in0 > np.float32(s0)).astype(np.float32)
        m = (in0 - s).astype(np.float32)
        return (m * np.float32(s1)).astype(np.float32) + in1

    op = dve_ops.DveOp(
        "LIF_STEP_ANT",
        Spec(body=(Src0 - (Src0 > C0)) * C1 + Src1, reference=_ref),
        subdim=False,
        uops_sha={"v3": "8c1c8b30d434ec6b"},
    )
    dve_ops.OPS.append(op)
    dve_ops._SUB_OPCODE_FOR_NAME[op.name] = (
        dve_ops._CUSTOM_DVE_ROW_BASE + len(dve_ops.OPS) - 1
    )
    dve_ops.CUSTOM_DVE_SPECS[op.name] = op.spec
    return op


def _register_lif2_op():
    """Register LIF_STEP2_ANT: hand-built 2-timestep fused LIF op.

    One instruction advances the chain TWO steps:
        v  = (u - (u > th)) * beta + x1     (= u_{t+1})
        u2 = (v - (v > th)) * beta + x2     (= u_{t+2})
    in0 = u [P, N] (consumed every 2nd cycle), in1 = x [P, N, 2],
    out = [P, N, 2] (v, u2). 3-uOp FSM alternating per element; uOp B
    reads v via same-stage CURR_ALU_OUT feedback. Raw uOps are injected
    via dve_ops._COMPILE_CACHE (the Spec-DSL lower() cannot express
    multi-rate FSMs); CoreSim uses the numpy reference below.
    HW-verified bit-exact vs two 1-step ops.
    """
    from concourse import dve_ops
    from concourse.dve_spec import Spec, Src0, Src1, C0, C1
    from concourse.dve_uop import (
        AluInp, AluOp, DveOpSpec, InpSel, OutPath, OutSel, Trigger,
        UopConfig,
    )

    NAME = "LIF_STEP2_ANT"
    for op in dve_ops.OPS:
        if op.name == NAME:
            return op

    def _ref2(in0, in1, s0, s1, imm2):
        th = np.float32(s0) if np.isscalar(s0) else np.asarray(s0, np.float32)
        be = np.float32(s1) if np.isscalar(s1) else np.asarray(s1, np.float32)

        def step(u, x):
            s = (u > th).astype(np.float32)
            m = (u - s).astype(np.float32)
            return (m * be).astype(np.float32) + x

        v = step(np.asarray(in0, np.float32),
                 np.asarray(in1[..., 0], np.float32))
        u2 = step(v, np.asarray(in1[..., 1], np.float32))
        return np.stack([v, u2], axis=-1)

    def _mk_uop(kind, nxt):
        u = UopConfig()
        # lanes: 0=u (A only), 1=threshold, 2=beta, 3=x
        if kind == "A":
            u.enable_input(InpSel.SRC_0, 1)
        u.enable_input(InpSel.CONST_0, 2)
        u.enable_input(InpSel.CONST_1, 3)
        u.enable_input(InpSel.SRC_1, 4)
        lanes = (0, 1, 2, 3) if kind == "A" else (1, 2, 3)
        dp = u.datapath_config
        for k in range(8):
            dp[k].pass_through_delay(*lanes)
        if kind == "A":
            dp[0].enable_alu(AluOp.IS_LT, AluInp.PREV_DELAY_1,
                             AluInp.PREV_DELAY_0)
            dp[1].enable_alu(AluOp.SUBTRACT, AluInp.PREV_DELAY_0,
                             AluInp.PREV_ALU_OUT)
            dp[2].enable_alu(AluOp.MULTIPLY, AluInp.PREV_ALU_OUT,
                             AluInp.PREV_DELAY_2)
            dp[3].enable_alu(AluOp.ADD, AluInp.PREV_ALU_OUT,
                             AluInp.PREV_DELAY_3)
            for k in range(4, 8):
                dp[k].pass_through_alu()
            u.require_inp0 = 1
            u.require_inp1 = 1
            u.trigger = (Trigger.COUNT, Trigger.NONE, Trigger.NONE)
            u.next_uop = (nxt, 0, 0)
            u.repeat_count = 1
        else:
            dp[4].enable_alu(AluOp.IS_LT, AluInp.PREV_DELAY_1,
                             AluInp.CURR_ALU_OUT)
            dp[5].enable_alu(AluOp.SUBTRACT, AluInp.CURR_ALU_OUT,
                             AluInp.PREV_ALU_OUT)
            dp[6].enable_alu(AluOp.MULTIPLY, AluInp.PREV_ALU_OUT,
                             AluInp.PREV_DELAY_2)
            dp[7].enable_alu(AluOp.ADD, AluInp.PREV_ALU_OUT,
                             AluInp.PREV_DELAY_3)
            u.require_inp0 = 0
            u.require_inp1 = 1
            u.trigger = (Trigger.SRC_TENSOR_DONE, Trigger.COUNT,
                         Trigger.NONE)
            u.next_uop = (0, nxt, 0)
            u.repeat_count = 1
        u.enable_output(OutSel.ALU_OUT, OutPath.WR0_LO)
        return u

    op = dve_ops.DveOp(
        NAME,
        # Dummy body (never lowered — compile cache pre-filled below).
        Spec(body=(Src0 - (Src0 > C0)) * C1 + Src1, reference=_ref2),
        subdim=False,
        uops_sha={},
    )
    dve_ops.OPS.append(op)
    dve_ops._SUB_OPCODE_FOR_NAME[NAME] = (
        dve_ops._CUSTOM_DVE_ROW_BASE + len(dve_ops.OPS) - 1
    )
    dve_ops.CUSTOM_DVE_SPECS[NAME] = op.spec
    # uops[0]=A entry, [1]=B, [2]=A loop (next_uop 0 means IDLE/exit,
    # so the A<->B loop runs over indices 1/2).
    raw = DveOpSpec(
        name=NAME,
        opcode=dve_ops.get_dve_sub_opcode(NAME),
        uops=[_mk_uop("A", 1), _mk_uop("B", 2), _mk_uop("A", 1)],
        rd1_en=True,
    )
    raw.validate("v3")
    dve_ops._COMPILE_CACHE[(NAME, "v3")] = raw
    return op


LIF_OP = _register_lif_op()
LIF2_OP = _register_lif2_op()

# --------------------------------------------------------------------------
# Problem geometry (hardcoded per contract).
# --------------------------------------------------------------------------
B, T, D = 16, 2048, 1024
N_CORES = 8
SEG = T // N_CORES          # 256 output steps per core
W = 128                     # warm-up steps (state resync from zero)
TSEG = SEG + W              # 384 sequential steps per core
P = 128                     # SBUF partitions
FD = (B * D) // P           # 128 free elems per step tile
EPP = D // FD               # 8 partitions per batch row
BETA = 0.9
F32 = mybir.dt.float32
F16 = mybir.dt.float16
OUT_DT = mybir.dt.float8e4  # sign(1-u) in {-1,0,1} — exact in fp8e4
Op = mybir.AluOpType
WH = 64                     # warm-up steps shipped as fp16 (first 2 blocks)


def build_program(K: int = 32, h: float = 0.0, reps: int = 1,
                  interleave: int = 2, w: int = W, unroll: int = 16,
                  fp16_wh: bool = True, xbufs: int = 4):
    """Single-core Bass/Tile program (same program on all cores).

    Core inputs: xh [P, WH, FD] fp16 (fp16 warm-up prefix, if fp16_wh)
    and x [P, tseg-WH, FD] fp32; output s [P, SEG, FD] fp8 holding
    sign(1-u) (host maps <0 -> spike).
    reps > 1 wraps the body in a hardware loop for wall-clock-slope
    timing (the computation is idempotent); unroll bodies per iteration.
    """
    tseg = SEG + w
    assert tseg % K == 0 and w % K == 0 and K % 2 == 0
    nblk = tseg // K
    wblk = w // K
    wh = WH if (fp16_wh and w >= WH) else 0
    whblk = wh // K
    assert wh % K == 0
    nc = bacc.Bacc("TRN2", target_bir_lowering=False, debug=False)
    if wh:
        xh_d = nc.dram_tensor("xh", [P, wh, FD], F16, kind="ExternalInput")
        xh_ap = xh_d.ap()
    x_d = nc.dram_tensor("x", [P, tseg - wh, FD], F32, kind="ExternalInput")
    s_d = nc.dram_tensor("s", [P, SEG, FD], OUT_DT, kind="ExternalOutput")
    x_ap = x_d.ap()
    s_ap = s_d.ap()
    sub = FD // interleave

    with tile.TileContext(nc) as tc:

        def body(rep, xp, up, sp):
            X = [None] * nblk
            U = [None] * nblk
            S = [None] * nblk

            def load(bb, split=1):
                dt = F16 if bb < whblk else F32
                X[bb] = xp.tile([P, K * FD], dt, name=f"x{rep}_{bb}",
                                tag="x")
                if bb < whblk:
                    src_ap, ofs = xh_ap, bb * K
                else:
                    src_ap, ofs = x_ap, bb * K - wh
                for q in range(split):
                    ks, ke = q * K // split, (q + 1) * K // split
                    src = src_ap[:, ofs + ks:ofs + ke, :].rearrange(
                        "p k j -> p (k j)")
                    nc.sync.dma_start(
                        out=X[bb][:, ks * FD:ke * FD], in_=src)
                if h != 0.0:
                    nc.vector.tensor_scalar(X[bb][:, :], X[bb][:, :],
                                            float(h), None, Op.add)

            def extract(bb, c0=0, c1=None):
                # One ACT pass: sign(1-u) in {-1,0,+1}; host decodes
                # (val < 0) -> spike. Keeps ACT (~3.6us/blk) well under
                # DVE (~6.2us/blk) so extraction never backlogs.
                c1 = K if c1 is None else c1
                if S[bb] is None:
                    S[bb] = sp.tile([P, K * FD], OUT_DT,
                                    name=f"s{rep}_{bb}", tag="s")
                cs, ce = c0 * FD, c1 * FD
                nc.scalar.activation(
                    S[bb][:, cs:ce], U[bb][:, cs:ce],
                    mybir.ActivationFunctionType.Sign, bias=1.0, scale=-1.0)
                dst = s_ap[:, (bb - wblk) * K + c0:(bb - wblk) * K + c1,
                           :].rearrange("p k j -> p (k j)")
                # Stores ride the ACT HWDGE ring so loads (SP ring)
                # never queue behind them.
                nc.scalar.dma_start(out=dst, in_=S[bb][:, cs:ce])

            load(0, split=2)
            load(1)
            U[0] = up.tile([P, K * FD], F32, name=f"u{rep}_0", tag="u")

            def step1(bb, k, sbb, sk, src=None):
                # u col (bb,k) = one LIF step from u col (sbb,sk); src
                # overrides the membrane input AP (first step reads x_0
                # directly: u_0 = x_0 since beta*0 + x_0 == x_0, so no
                # copy into U is needed — U col 0 is never read again).
                for i in range(interleave):
                    lo, hi = i * sub, (i + 1) * sub
                    in0 = (src if src is not None else
                           U[sbb][:, sk * FD:(sk + 1) * FD])
                    nc.vector._custom_dve(
                        LIF_OP,
                        out=U[bb][:, k * FD + lo:k * FD + hi],
                        in0=in0[:, lo:hi],
                        in1=X[bb][:, k * FD + lo:k * FD + hi],
                        s0=1.0, s1=BETA)

            def step2(bb, k, sbb, sk):
                # u cols (bb,k) and (bb,k+1) = one fused 2-step op from
                # u col (sbb,sk); x/out as [P, j, c] strided views (c =
                # step column, iterated innermost = the op's A/B element
                # order).
                for i in range(interleave):
                    lo, hi = i * sub, (i + 1) * sub
                    out2 = U[bb][:, k * FD:(k + 2) * FD].rearrange(
                        "p (c j) -> p j c", c=2)[:, lo:hi, :]
                    xin2 = X[bb][:, k * FD:(k + 2) * FD].rearrange(
                        "p (c j) -> p j c", c=2)[:, lo:hi, :]
                    nc.vector._custom_dve(
                        LIF2_OP, out=out2,
                        in0=U[sbb][:, sk * FD + lo:sk * FD + hi],
                        in1=xin2, s0=1.0, s1=BETA)

            step1(0, 1, 0, 0, src=X[0][:, 0:FD])
            for t in range(2, tseg, 2):
                bb, k = divmod(t, K)
                if k == 0:
                    if bb + 1 < nblk:
                        load(bb + 1)
                    U[bb] = up.tile([P, K * FD], F32, name=f"u{rep}_{bb}",
                                    tag="u")
                sbb, sk = divmod(t - 1, K)
                step2(bb, k, sbb, sk)
                if bb >= wblk:
                    # Last block: extract in quarters (first three fire
                    # mid-block) so the rep-end ACT+store tail is small.
                    last = bb == nblk - 1
                    if last:
                        q = K // 4
                        if k % q == q - 2:
                            c0 = (k + 2) - q
                            extract(bb, c0, c0 + q)
                    elif k == K - 2:
                        extract(bb, 0, K)

        def emit_group(n_iters, bodies):
            if n_iters <= 0 or bodies <= 0:
                return
            with ExitStack() as ctx:
                if n_iters > 1:
                    ctx.enter_context(tc.For_i(0, n_iters, 1))
                xp = ctx.enter_context(tc.tile_pool(name="xp", bufs=xbufs))
                up = ctx.enter_context(tc.tile_pool(name="up", bufs=3))
                sp = ctx.enter_context(tc.tile_pool(name="sp", bufs=4))
                for r in range(bodies):
                    body(r, xp, up, sp)

        if reps <= unroll:
            emit_group(1, reps)
        else:
            emit_group(reps // unroll, unroll)
            emit_group(1, reps % unroll)

    nc.compile()
    return nc


@functools.lru_cache(maxsize=2)
def _get_program(h: float):
    return build_program(h=h)


# --------------------------------------------------------------------------
# Host-side sharding / layout
# --------------------------------------------------------------------------

def to_pmaj(xs: np.ndarray) -> np.ndarray:
    """[B, t, D] -> [P, t, FD] with p = b*EPP + (d>>7), j = d&127."""
    t = xs.shape[1]
    return np.ascontiguousarray(
        xs.reshape(B, t, EPP, FD).transpose(0, 2, 1, 3).reshape(P, t, FD)
    )


def from_pmaj(sp_: np.ndarray) -> np.ndarray:
    """[P, t, FD] -> [B, t, D] (inverse of to_pmaj)."""
    t = sp_.shape[1]
    return sp_.reshape(B, EPP, t, FD).transpose(0, 2, 1, 3).reshape(B, t, D)


def decode_spikes(s_raw: np.ndarray) -> np.ndarray:
    """fp8 sign(1-u) -> spikes: val<0 means u>1 (exact)."""
    return (np.asarray(s_raw, np.float32) < 0).astype(np.float32)


def split_inputs(xw_core: np.ndarray, h: float) -> dict:
    """[B, TSEG, D] core window -> {xh: fp16 pmaj, x: fp32 pmaj}."""
    xh = to_pmaj(xw_core[:, :WH]).astype(np.float16)
    x = to_pmaj(xw_core[:, WH:])
    return {"xh": xh, "x": x}


def _shard_inputs(x: np.ndarray, h: float) -> list[dict]:
    """Per-core time slices with W warm-up steps prepended. Core 0's pad
    is -h so after the on-device +h its effective warm-up input is
    exactly zero (zero input keeps zero state -> core 0 is exact)."""
    pad = np.full((B, W, D), np.float32(-h), np.float32)
    xw = np.concatenate([pad, x], axis=1)  # [B, W+T, D]
    return [
        split_inputs(xw[:, c * SEG:c * SEG + TSEG], h)
        for c in range(N_CORES)
    ]


def kernel(x: np.ndarray, homeo_i: np.ndarray) -> np.ndarray:
    x = np.ascontiguousarray(np.asarray(x, dtype=np.float32))
    h = float(np.asarray(homeo_i).reshape(-1)[0])
    assert x.shape == (B, T, D), x.shape
    nc = _get_program(h)
    res = run_bass_kernel_spmd(nc, _shard_inputs(x, h),
                               list(range(N_CORES)))
    out = np.concatenate(
        [from_pmaj(decode_spikes(res.results[c]["s"]))
         for c in range(N_CORES)], axis=1)
    return out
